# revision 23
# baseline (speedup 1.0000x reference)
"""GCN classifier kernel for Trainium2, 8 NeuronCores — v2.

Strategy: graph-aligned node sharding (each core owns the nodes of 64 of the
512 graphs, padded to NPC nodes). Edges bucketed by (src-quarter s, dst-block
blk) cells, sorted by src within a cell; cells packed into merged dma_gather
calls of <=2048 idxs (ucode limit) grouped over consecutive dst blocks so the
SWDGE prep overhead amortizes. Aggregation per dst block:

  agg[dst_blk] = sum_chunks mask_w^T @ msg   +   I @ slab[dst_blk]

where mask_w[e, dst] = ew[e] * (dstloc[e] == dst) is built in ONE fused
tensor_scalar (is_equal, mult) per 128-edge chunk, and msg is the raw gathered
bf16 node-state row (no separate edge-weight multiply). The degree pass uses
the same fused masks against a ones column accumulated into one PSUM tile.
Node state (h, hsum, pooling transpose) stays SBUF-resident; LayerNorm /
residual epilogues split across ACT and DVE with zero-valued params skipped.
Mean/max pooling via per-block PE transposes into a resident xmT tile +
per-graph free-axis reduces (no gathers); head replicated after an AllGather.
"""

import os
import sys
import types

sys.path.insert(0, "/opt/trn_rl_repo")

import numpy as np
import ml_dtypes

BF16 = ml_dtypes.bfloat16

# Shim antenv.axon_hooks (missing in this image) so trace=True can work.
try:
    import antenv.axon_hooks  # noqa: F401
except ImportError:
    try:
        from trn_agent_boot.trn_boot import _ntff_profile_via_ctypes
        _hook = _ntff_profile_via_ctypes('/opt/axon/libaxon_pjrt.so')
    except Exception:
        _hook = None
    _mod = types.ModuleType('antenv.axon_hooks')
    _mod.get_axon_ntff_profile_hook = lambda: _hook
    sys.modules['antenv.axon_hooks'] = _mod

import concourse.bacc as bacc
import concourse.mybir as mybir
import concourse.tile as tile
import concourse.bass_utils as bass_utils

# No bucket access in this container.
bass_utils.upload_artifacts = lambda tmpdir: tmpdir

F32 = mybir.dt.float32
BF = mybir.dt.bfloat16
I16 = mybir.dt.int16
AOP = mybir.AluOpType
ACTF = mybir.ActivationFunctionType
AXX = mybir.AxisListType.X

NCORES = 8
H = 128        # hidden channels
IN = 96        # in channels
ED = 8         # edge dim
NCLS = 100     # classes
L = 4          # layers
NGRAPH = 512   # graphs
EPS_LN = 1e-5
MAXG = 2048    # dma_gather num_idxs hard limit (3072+ wedges the device)
PHASE = int(os.environ.get('KPHASE', '9'))  # debug bisect: 1=mlp+deg 2=+hws0 3=+AG 4=+layers
NSR = 4        # src ranges (int16 idx limit)


def _split_waits(nc, max_waits=1):
    """This container's walrus rejects >1 sync wait per instruction; move
    extra waits onto preceding NOPs on the same engine."""
    n = 0
    for f in nc.m.functions:
        for bb in f.blocks:
            new_list = []
            for ins in bb.instructions:
                si = ins.sync_info
                if si and si.on_wait and len(si.on_wait) > max_waits:
                    waits = list(si.on_wait)
                    extra, keep = waits[:-max_waits], waits[-max_waits:]
                    for i, w in enumerate(extra):
                        nop = mybir.InstNoOp(name=f"{ins.name}-ws{i}", ins=[], outs=[])
                        nop.engine = ins.engine
                        nop.sync_info = mybir.SyncInfo(on_wait=[w], on_update=[])
                        new_list.append(nop)
                        n += 1
                    si.on_wait = keep
                new_list.append(ins)
            bb.instructions[:] = new_list
    return n


def _fix_act_tables(nc, set_id=6):
    """All activation funcs used here live in act table 6; unify and dedupe
    the per-func table loads (saves ~27us per reload)."""
    removed = 0
    for f in nc.m.functions:
        for bb in f.blocks:
            new_list = []
            loaded = False
            for ins in bb.instructions:
                if isinstance(ins, mybir.InstLoadActFuncSet):
                    ins.act_func_set_id = set_id
                    si = ins.sync_info
                    has_sync = si and (si.on_wait or si.on_update)
                    if loaded and not has_sync:
                        removed += 1
                        continue
                    if loaded and has_sync:
                        nop = mybir.InstNoOp(name=ins.name + "-actdedup", ins=[], outs=[])
                        nop.engine = ins.engine
                        nop.sync_info = si
                        new_list.append(nop)
                        removed += 1
                        continue
                    loaded = True
                new_list.append(ins)
            bb.instructions[:] = new_list
    return removed


def _ru(x, m):
    return (x + m - 1) // m * m


def _wrap_idxs(idx):
    """[n] int -> [128, n//16] int16 SBUF wrap (i -> partition i%16, col i//16),
    replicated over the 8 gpsimd cores."""
    n = len(idx)
    assert n % 16 == 0
    a = np.asarray(idx, np.int16).reshape(n // 16, 16).T.copy()
    return np.tile(a, (8, 1))


class Plan:
    pass


def make_plan(x, edge_index, batch, edge_attr):
    N = x.shape[0]
    E = edge_index.shape[1]
    p = Plan()
    p.N, p.E = N, E

    batch = np.asarray(batch, np.int64)
    src = np.asarray(edge_index[0], np.int64)
    dst = np.asarray(edge_index[1], np.int64)

    # graph-aligned node -> core assignment
    node_start = np.searchsorted(batch, np.arange(NGRAPH + 1))  # [513]
    gpc = NGRAPH // NCORES
    core_start = node_start[::gpc][:NCORES].astype(np.int64)
    core_end = np.append(core_start[1:], N).astype(np.int64)
    core_cnt = core_end - core_start
    NPC = max(256, _ru(int(core_cnt.max()), 128))
    NBLK = NPC // 128
    NPAD = NCORES * NPC
    SR = NPAD // NSR
    assert SR <= 32767, (NPC, SR)
    p.NPC, p.NBLK, p.NPAD, p.SR = NPC, NBLK, NPAD, SR
    p.core_start, p.core_cnt = core_start, core_cnt

    # original node id -> padded id (nodes already grouped by graph)
    owner_of_node = np.searchsorted(core_start, np.arange(N), side='right') - 1
    padded = owner_of_node * NPC + (np.arange(N) - core_start[owner_of_node])

    src_p = padded[src]
    dst_p = padded[dst]
    e_owner = dst_p // NPC
    dst_loc = dst_p - e_owner * NPC
    blk = dst_loc >> 7
    srange = src_p // SR
    cell = srange * NBLK + blk          # s-major cell id
    NCELL = NSR * NBLK

    # sort: per core, s-major, block-minor, src ascending within a cell
    order = np.lexsort((src_p, cell, e_owner))
    src_p, dst_loc, cell, e_owner = (src_p[order], dst_loc[order],
                                     cell[order], e_owner[order])
    ea_perm = np.asarray(edge_attr, np.float32)[order]

    counts = np.zeros((NCORES, NCELL), np.int64)
    for r in range(NCORES):
        m = e_owner == r
        counts[r] = np.bincount(cell[m], minlength=NCELL)
    core_off = np.searchsorted(e_owner, np.arange(NCORES + 1))
    core_cell_off = np.zeros((NCORES, NCELL + 1), np.int64)
    for r in range(NCORES):
        core_cell_off[r, 0] = core_off[r]
        core_cell_off[r, 1:] = np.cumsum(counts[r]) + core_off[r]

    cell_num = _ru(counts.max(axis=0), 128)  # padded slot count per cell
    assert cell_num.min() >= 128, "empty cell unsupported"

    # groups of consecutive blocks; per group one gather call per srange,
    # each call <= MAXG idxs
    groups = []
    b0 = 0
    while b0 < NBLK:
        b1 = b0 + 1
        while b1 < NBLK:
            ok = all(int(cell_num[s * NBLK + b0:s * NBLK + b1 + 1].sum()) <= MAXG
                     for s in range(NSR))
            if not ok:
                break
            b1 += 1
        groups.append((b0, b1))
        b0 = b1
    # layout: s-major over cells; chunk (dl) and idx offsets follow
    cell_dl = np.zeros(NCELL + 1, np.int64)
    cell_dl[1:] = np.cumsum(cell_num // 128)
    NDL = int(cell_dl[-1])
    p.NDL = NDL
    p.NIDX16 = NDL * 8           # idx cols of 16 = slots/16

    # call descriptors per (group, s)
    p.groups = []
    for (b0, b1) in groups:
        calls = []
        for s in range(NSR):
            c0, c1 = s * NBLK + b0, s * NBLK + b1
            dl0, dl1 = int(cell_dl[c0]), int(cell_dl[c1])
            calls.append(dict(s=s, dl_off=dl0, nch=dl1 - dl0,
                              num=(dl1 - dl0) * 128, idx_off=dl0 * 8))
        p.groups.append(dict(b0=b0, b1=b1, calls=calls))
    p.cell_dl = cell_dl

    # per-core tensors
    p.idx_all = np.zeros((NCORES, 128, p.NIDX16), np.int16)
    p.dstloc_all = np.full((NCORES, 128, NDL), 255.0, np.float32)
    p.eaT_all = np.zeros((NCORES, 8, NDL * 128), BF16)
    for r in range(NCORES):
        for c in range(NCELL):
            s = c // NBLK
            nslots = int(cell_num[c])
            if nslots == 0:
                continue
            a0, a1 = core_cell_off[r, c], core_cell_off[r, c + 1]
            n_real = int(a1 - a0)
            dl0 = int(cell_dl[c])
            idx = np.zeros(nslots, np.int64)
            if n_real:
                idx[:n_real] = src_p[a0:a1] - s * SR
                idx[n_real:] = idx[0]          # pad: repeat a real row
            p.idx_all[r, :, dl0 * 8:dl0 * 8 + nslots // 16] = _wrap_idxs(idx)
            dl = np.full(nslots, 255.0, np.float32)
            if n_real:
                dl[:n_real] = (dst_loc[a0:a1] - (c % NBLK) * 128).astype(np.float32)
            p.dstloc_all[r, :, dl0:dl0 + nslots // 128] = \
                dl.reshape(nslots // 128, 128).T
            if n_real:
                ea = np.zeros((nslots, ED), np.float32)
                ea[:n_real] = ea_perm[a0:a1]
                p.eaT_all[r, :, dl0 * 128:dl0 * 128 + nslots] = ea.T.astype(BF16)

    # x slab, transposed [96, NPC] per core
    p.xT = np.zeros((NCORES, IN, NPC), np.float32)
    xf = np.asarray(x, np.float32)
    for r in range(NCORES):
        p.xT[r, :, :core_cnt[r]] = xf[core_start[r]:core_end[r]].T

    # pooling. Mean: per-block graph-membership one-hot [node, graph] fed to
    # PE (SPMD-safe; membership is per-core data). Max: per-graph padded
    # gather of xm rows (pad = repeat first node) + transpose + free-reduce.
    gcnt = (node_start[1:] - node_start[:-1]).astype(np.int64)
    MAXN = max(128, _ru(int(gcnt.max()), 128))
    assert MAXN <= MAXG
    p.MAXN = MAXN
    p.gmemb = np.zeros((NCORES, NPC, gpc), BF16)
    p.pmax_idx = np.zeros((NCORES, 128, gpc * MAXN // 16), np.int16)
    p.invcntc = np.zeros((NCORES, gpc, 1), np.float32)
    for r in range(NCORES):
        mi = []
        for j in range(gpc):
            gid = r * gpc + j
            a = int(node_start[gid] - core_start[r])
            n = int(gcnt[gid])
            p.gmemb[r, a:a + n, j] = 1.0
            ids = np.arange(a, a + n)
            pad = MAXN - n
            mi.append(np.concatenate([ids, np.full(pad, ids[0] if n else 0)]))
            p.invcntc[r, j, 0] = 1.0 / max(n, 1)
        p.pmax_idx[r] = _wrap_idxs(np.concatenate(mi))
    return p


def build_nc(p, w):
    nc = bacc.Bacc("TRN2", num_devices=NCORES, detect_race_conditions=False,
                   num_swdge_queues=4)
    NPC, NBLK, NPAD, SR = p.NPC, p.NBLK, p.NPAD, p.SR
    NDL = p.NDL
    GPC = NGRAPH // NCORES

    t_xT = nc.dram_tensor("xT", [IN, NPC], F32, kind="ExternalInput")
    t_idx = nc.dram_tensor("idx", [128, p.NIDX16], I16, kind="ExternalInput")
    t_dstloc = nc.dram_tensor("dstloc", [128, NDL], F32, kind="ExternalInput")
    t_eaT = nc.dram_tensor("eaT", [8, NDL * 128], BF, kind="ExternalInput")
    t_invcntc = nc.dram_tensor("invcntc", [GPC, 1], F32, kind="ExternalInput")
    t_gmemb = nc.dram_tensor("gmemb", [NPC, GPC], BF, kind="ExternalInput")
    t_pmax = nc.dram_tensor("pmax", [128, GPC * p.MAXN // 16], I16,
                            kind="ExternalInput")
    t_W0 = nc.dram_tensor("W0", [IN, H], F32, kind="ExternalInput")
    t_resW = nc.dram_tensor("resW", [IN, H], F32, kind="ExternalInput")
    t_Wk = nc.dram_tensor("Wk", [H, 3 * H], BF, kind="ExternalInput")
    t_rows = nc.dram_tensor("rows", [128, 16 * H], F32, kind="ExternalInput")
    t_eeW1 = nc.dram_tensor("eeW1", [ED, H], BF, kind="ExternalInput")
    t_eeW2 = nc.dram_tensor("eeW2", [H, 1], BF, kind="ExternalInput")
    t_eeb1 = nc.dram_tensor("eeb1", [H, 1], F32, kind="ExternalInput")
    t_hW1 = nc.dram_tensor("hW1", [H, 2 * H], F32, kind="ExternalInput")
    t_hW2 = nc.dram_tensor("hW2", [H, NCLS], F32, kind="ExternalInput")
    t_iota_bf = nc.dram_tensor("iota_bf", [128, 128], BF, kind="ExternalInput")
    t_ident_bf = nc.dram_tensor("ident_bf", [128, 128], BF, kind="ExternalInput")
    t_ident_f = nc.dram_tensor("ident_f", [128, 128], F32, kind="ExternalInput")
    t_out = nc.dram_tensor("out", [NGRAPH, NCLS], F32, kind="ExternalOutput")

    ee_b2 = float(np.asarray(w['ee_b2']).reshape(-1)[0])
    # zero/identity param detection (baked per input values)
    any_cb = any(np.any(np.asarray(w[f'cb{i}'])) for i in range(4))
    any_g = any(np.any(np.asarray(w[f'g{i}']) != 1.0) for i in range(4))
    any_be = any(np.any(np.asarray(w[f'be{i}'])) for i in range(4))
    any_resb = bool(np.any(np.asarray(w['res_b'])))
    any_hb1 = bool(np.any(np.asarray(w['hb1'])))
    any_hb2 = bool(np.any(np.asarray(w['hb2'])))

    gq_counter = [0]

    def next_q():
        q = gq_counter[0] % 4
        gq_counter[0] += 1
        return q

    # first/last chunk bookkeeping for the deg accumulation per block
    blk_nch = np.zeros(NBLK, np.int64)
    for s in range(NSR):
        for b in range(NBLK):
            c = s * NBLK + b
            blk_nch[b] += int(p.cell_dl[c + 1] - p.cell_dl[c])

    with tile.TileContext(nc) as tc:
        with (
            tc.tile_pool(name="const", bufs=1) as cp,
            tc.tile_pool(name="dram", bufs=1, space="DRAM") as dp,
            tc.tile_pool(name="ea", bufs=2) as eap,
            tc.tile_pool(name="eh", bufs=2) as ehp,
            tc.tile_pool(name="msg", bufs=8) as msgp,
            tc.tile_pool(name="mask", bufs=8) as mkp,
            tc.tile_pool(name="blk", bufs=2) as bp,
            tc.tile_pool(name="pag", bufs=2, space="PSUM") as pag,
            tc.tile_pool(name="pmm", bufs=2, space="PSUM") as pmm,
            tc.tile_pool(name="psm", bufs=2, space="PSUM") as psm,
            tc.tile_pool(name="pdg", bufs=1, space="PSUM") as pdg,
        ):
            # ---------- resident tiles ----------
            def load_const(t, shape, dtype, tag):
                tl = cp.tile(shape, dtype, tag=tag)
                nc.sync.dma_start(tl[:], t[:])
                return tl

            idx_sb = load_const(t_idx, [128, p.NIDX16], I16, "idx_sb")
            dstloc = load_const(t_dstloc, [128, NDL], F32, "dstloc")
            W0 = load_const(t_W0, [IN, H], F32, "W0")
            resW = load_const(t_resW, [IN, H], F32, "resW")
            Wk = load_const(t_Wk, [H, 3 * H], BF, "Wk")
            eeW1 = load_const(t_eeW1, [ED, H], BF, "eeW1")
            eeW2 = load_const(t_eeW2, [H, 1], BF, "eeW2")
            eeb1 = load_const(t_eeb1, [H, 1], F32, "eeb1")
            hW1 = load_const(t_hW1, [H, 2 * H], F32, "hW1")
            hW2 = load_const(t_hW2, [H, NCLS], F32, "hW2")
            iota_bf = load_const(t_iota_bf, [128, 128], BF, "iota_bf")
            ident_bf = load_const(t_ident_bf, [128, 128], BF, "ident_bf")
            ident_f = load_const(t_ident_f, [128, 128], F32, "ident_f")
            invcntc = load_const(t_invcntc, [GPC, 1], F32, "invcntc")
            pmax_sb = load_const(t_pmax, [128, GPC * p.MAXN // 16], I16, "pmax")
            rows = load_const(t_rows, [128, 16 * H], F32, "rows")

            b2col = cp.tile([128, 1], F32, tag="b2col")
            nc.vector.memset(b2col[:], ee_b2)
            ones_bf = cp.tile([128, 1], BF, tag="ones_bf")
            nc.vector.memset(ones_bf[:], 1.0)
            epscol = cp.tile([128, 1], F32, tag="epscol")
            nc.vector.memset(epscol[:], EPS_LN)

            ew = cp.tile([128, NDL], F32, tag="ew")
            dis = cp.tile([128, NBLK], F32, tag="dis")
            slab = cp.tile([128, NPC], BF, tag="slab")
            h_prev = cp.tile([128, NPC], BF, tag="h_prev")
            hsum = cp.tile([128, NPC], BF, tag="hsum")
            gmaxT = cp.tile([128, GPC], F32, tag="gmaxT")

            # ---------- DRAM scratch ----------
            tables = [dp.tile([NPAD, H], BF, addr_space="Shared", tag=f"table{_k}",
                              name=f"table{_k}") for _k in range(L)]
            slab_hbm = dp.tile([NPC, H], BF)
            res0_hbm = dp.tile([NPC, H], F32)
            pool_tab = dp.tile([NPC, H], BF)
            gpart = dp.tile([GPC, 2 * H], F32)
            gfull = dp.tile([NGRAPH, 2 * H], F32, addr_space="Shared")

            # =============== edge MLP + deg (pipelined per call) ============
            deg_ps = pdg.tile([128, NBLK], F32, tag="deg")
            blk_seen = np.zeros(NBLK, np.int64)
            for grp in (p.groups if PHASE >= 1 else []):
                for call in grp['calls']:
                    nch, dl0 = call['nch'], call['dl_off']
                    if nch == 0:
                        continue
                    cols = nch * 128
                    ea_t = eap.tile([8, MAXG], BF, tag="ea")
                    nc.sync.dma_start(ea_t[:, :cols],
                                      t_eaT[:, dl0 * 128:dl0 * 128 + cols])
                    eh = ehp.tile([128, MAXG], BF, tag="eh")
                    for c0 in range(0, cols, 512):
                        c1 = min(c0 + 512, cols)
                        eh_ps = pmm.tile([128, 512], F32, tag="mm")
                        nc.tensor.matmul(eh_ps[:, :c1 - c0], lhsT=eeW1[:, :],
                                         rhs=ea_t[:, c0:c1], start=True, stop=True)
                        nc.vector.tensor_scalar(eh[:, c0:c1], eh_ps[:, :c1 - c0],
                                                eeb1[:, :], 0.0, AOP.add, AOP.max)
                    z_ps = psm.tile([128, 16], F32, tag="sm")
                    for c in range(nch):
                        nc.tensor.matmul(z_ps[:, c:c + 1],
                                         lhsT=eh[:, c * 128:(c + 1) * 128],
                                         rhs=eeW2[:, :], start=True, stop=True,
                                         skip_group_check=True)
                    # softplus(z + b2) + 1e-4
                    ez = bp.tile([128, 16], F32, tag="ez")
                    nc.scalar.activation(ez[:, :nch], z_ps[:, :nch], ACTF.Exp,
                                         bias=b2col[:, :], scale=1.0)
                    lg = bp.tile([128, 16], F32, tag="lg")
                    nc.scalar.activation(lg[:, :nch], ez[:, :nch], ACTF.Ln,
                                         bias=1.0, scale=1.0)
                    nc.vector.tensor_scalar(ew[:, dl0:dl0 + nch], lg[:, :nch],
                                            1e-4, None, AOP.add)
                # deg matmuls for this group's blocks (all 4 calls' ew ready)
                for b in range(grp['b0'], grp['b1']):
                    for s in range(NSR):
                        c = s * NBLK + b
                        dl0, dl1 = int(p.cell_dl[c]), int(p.cell_dl[c + 1])
                        for dl in range(dl0, dl1):
                            mk = mkp.tile([128, 128], BF, tag="mask")
                            nc.vector.tensor_scalar(mk[:], iota_bf[:, :],
                                                    dstloc[:, dl:dl + 1],
                                                    ew[:, dl:dl + 1],
                                                    AOP.is_equal, AOP.mult)
                            nc.tensor.matmul(
                                deg_ps[:, b:b + 1], lhsT=mk[:], rhs=ones_bf[:, :],
                                start=(blk_seen[b] == 0),
                                stop=(blk_seen[b] == blk_nch[b] - 1),
                                skip_group_check=True)
                            blk_seen[b] += 1
            # dis = (1 + deg)^-1/2
            if PHASE >= 1:
                lnd = bp.tile([128, NBLK], F32, tag="lnd")
                nc.scalar.activation(lnd[:], deg_ps[:], ACTF.Ln, bias=1.0, scale=1.0)
                nc.scalar.activation(dis[:], lnd[:], ACTF.Exp, bias=0.0, scale=-0.5)
            else:
                nc.vector.memset(ew[:], 0.01)
                nc.vector.memset(dis[:], 1.0)

            # =============== hws0 / res0 ===============
            if PHASE < 2:
                nc.vector.memset(slab[:], 0.0)
            for b in (range(NBLK) if PHASE >= 2 else []):
                bs = slice(b * 128, (b + 1) * 128)
                xtb = bp.tile([IN, 128], F32, tag="xtb")
                nc.sync.dma_start(xtb[:], t_xT[:, bs])
                hw_ps = pmm.tile([128, 512], F32, tag="mm")
                nc.tensor.matmul(hw_ps[:, :H], lhsT=xtb[:, :],
                                 rhs=W0[:, :], start=True, stop=True)
                nc.vector.tensor_scalar(slab[:, bs], hw_ps[:, :H],
                                        dis[:, b:b + 1], None, AOP.mult)
                rs_ps = pmm.tile([128, 512], F32, tag="mm")
                nc.tensor.matmul(rs_ps[:, :H], lhsT=xtb[:, :],
                                 rhs=resW[:, :], start=True, stop=True)
                r0 = bp.tile([128, H], F32, tag="r0")
                if any_resb:
                    nc.vector.tensor_tensor(out=r0[:], in0=rs_ps[:, :H],
                                            in1=rows[:, 12 * H:13 * H], op=AOP.add)
                else:
                    nc.vector.tensor_copy(r0[:], rs_ps[:, :H])
                nc.sync.dma_start(res0_hbm[bs, :], r0[:])
            nc.sync.dma_start(
                slab_hbm[:].rearrange("(b q) f -> q b f", q=128),
                slab[:].rearrange("p (b f) -> p b f", f=H))

            # =============== layers ===============
            gsum_ps = pdg.tile([GPC, H], F32, tag="gsum")
            for k in (range(L) if PHASE >= 3 else []):
                table = tables[k]
                nc.gpsimd.collective_compute(
                    "AllGather", AOP.bypass,
                    replica_groups=[list(range(NCORES))],
                    ins=[slab_hbm[:].opt()], outs=[table[:].opt()])

                for grp in (p.groups if PHASE >= 4 else []):
                    msgs = []
                    for call in grp['calls']:
                        nch = call['nch']
                        s = call['s']
                        msg = msgp.tile([128, MAXG // 128, 128], BF, tag="msg")
                        nc.gpsimd.dma_gather(
                            out_ap=msg[:, :nch, :],
                            in_ap=table[s * SR:(s + 1) * SR, :],
                            idxs_ap=idx_sb[:, call['idx_off']:
                                           call['idx_off'] + call['num'] // 16],
                            num_idxs=call['num'], num_idxs_reg=call['num'],
                            elem_size=H, queue_num=next_q())
                        msgs.append(msg)
                    for b in range(grp['b0'], grp['b1']):
                        bs = slice(b * 128, (b + 1) * 128)
                        agg = pag.tile([128, H], F32, tag="agg")
                        chunks = []
                        for s in range(NSR):
                            call = grp['calls'][s]
                            c = s * NBLK + b
                            dl0, dl1 = int(p.cell_dl[c]), int(p.cell_dl[c + 1])
                            for dl in range(dl0, dl1):
                                chunks.append((s, dl, dl - call['dl_off']))
                        for i, (s, dl, lc) in enumerate(chunks):
                            mk = mkp.tile([128, 128], BF, tag="mask")
                            nc.vector.tensor_scalar(mk[:], iota_bf[:, :],
                                                    dstloc[:, dl:dl + 1],
                                                    ew[:, dl:dl + 1],
                                                    AOP.is_equal, AOP.mult)
                            nc.tensor.matmul(agg[:], lhsT=mk[:],
                                             rhs=msgs[s][:, lc, :],
                                             start=(i == 0), stop=False,
                                             skip_group_check=True)
                        nc.tensor.matmul(agg[:], lhsT=ident_bf[:, :],
                                         rhs=slab[:, bs],
                                         start=(len(chunks) == 0), stop=True,
                                         skip_group_check=True)
                        # ---------- epilogue ----------
                        u = bp.tile([128, H], F32, tag="u")
                        nc.vector.tensor_scalar(u[:], agg[:], dis[:, b:b + 1],
                                                None, AOP.mult)
                        if any_cb:
                            nc.vector.tensor_tensor(
                                out=u[:], in0=u[:],
                                in1=rows[:, k * H:(k + 1) * H], op=AOP.add)
                        mu = bp.tile([128, 1], F32, tag="mu")
                        nc.vector.reduce_sum(mu[:], u[:], axis=AXX)
                        nc.vector.tensor_scalar(mu[:], mu[:], -1.0 / H, None,
                                                AOP.mult)
                        xc = bp.tile([128, H], F32, tag="xc")
                        nc.scalar.activation(xc[:], u[:], ACTF.Identity,
                                             bias=mu[:, :], scale=1.0)
                        sq = bp.tile([128, H], F32, tag="sq")
                        var = bp.tile([128, 1], F32, tag="var")
                        nc.scalar.activation(sq[:], xc[:], ACTF.Square,
                                             bias=0.0, scale=1.0, accum_out=var[:])
                        lnv = bp.tile([128, 1], F32, tag="lnv")
                        nc.scalar.activation(lnv[:], var[:], ACTF.Ln,
                                             bias=epscol[:, :], scale=1.0 / H)
                        inv = bp.tile([128, 1], F32, tag="inv")
                        nc.scalar.activation(inv[:], lnv[:], ACTF.Exp,
                                             bias=0.0, scale=-0.5)
                        y = bp.tile([128, H], F32, tag="y")
                        nc.scalar.mul(y[:], xc[:], inv[:, :])
                        if any_g:
                            nc.vector.tensor_tensor(
                                out=y[:], in0=y[:],
                                in1=rows[:, (4 + k) * H:(5 + k) * H], op=AOP.mult)
                        if any_be:
                            nc.vector.tensor_tensor(
                                out=y[:], in0=y[:],
                                in1=rows[:, (8 + k) * H:(9 + k) * H], op=AOP.add)
                        t_ = bp.tile([128, H], F32, tag="t_")
                        if k == 0:
                            res = bp.tile([128, H], F32, tag="res")
                            nc.sync.dma_start(res[:], res0_hbm[bs, :])
                            nc.vector.tensor_tensor(out=t_[:], in0=y[:],
                                                    in1=res[:], op=AOP.add)
                        else:
                            nc.vector.tensor_tensor(out=t_[:], in0=y[:],
                                                    in1=h_prev[:, bs], op=AOP.add)
                        # h (bf16) into h_prev
                        nc.scalar.activation(h_prev[:, bs], t_[:], ACTF.Relu,
                                             bias=0.0, scale=1.0)
                        if k == 0:
                            nc.vector.tensor_copy(hsum[:, bs], h_prev[:, bs])
                        else:
                            nc.vector.tensor_tensor(out=hsum[:, bs],
                                                    in0=hsum[:, bs],
                                                    in1=h_prev[:, bs], op=AOP.add)
                        if k < L - 1:
                            hT_ps = psm.tile([128, 128], BF, tag="sm")
                            nc.tensor.transpose(hT_ps[:], h_prev[:, bs],
                                                ident_bf[:, :])
                            hT = bp.tile([128, H], BF, tag="hT")
                            nc.vector.tensor_copy(hT[:], hT_ps[:])
                            hw_ps = pmm.tile([128, 512], F32, tag="mm")
                            nc.tensor.matmul(hw_ps[:, :H], lhsT=hT[:],
                                             rhs=Wk[:, k * H:(k + 1) * H],
                                             start=True, stop=True)
                            nc.vector.tensor_scalar(slab[:, bs], hw_ps[:, :H],
                                                    dis[:, b:b + 1], None, AOP.mult)
                        else:
                            xm = bp.tile([128, H], BF, tag="xm")
                            nc.vector.tensor_scalar(xm[:], hsum[:, bs], 0.25,
                                                    None, AOP.mult)
                            nc.sync.dma_start(pool_tab[bs, :], xm[:])
                            gmb = bp.tile([128, GPC], BF, tag="gmb")
                            nc.sync.dma_start(gmb[:], t_gmemb[bs, :])
                            nc.tensor.matmul(gsum_ps[:GPC, :], lhsT=gmb[:],
                                             rhs=xm[:],
                                             start=(b == 0), stop=(b == NBLK - 1),
                                             skip_group_check=True)
                if k < L - 1:
                    nc.sync.dma_start(
                        slab_hbm[:].rearrange("(b q) f -> q b f", q=128),
                        slab[:].rearrange("p (b f) -> p b f", f=H))

            # =============== pooling ===============
            if PHASE < 4:
                zrow = bp.tile([128, H], BF, tag="zrow")
                nc.vector.memset(zrow[:], 0.0)
                for b in range(NBLK):
                    nc.sync.dma_start(pool_tab[b * 128:(b + 1) * 128, :], zrow[:])
            # mean: finalize the membership-matmul accumulator
            gmean_sb = bp.tile([GPC, H], F32, tag="gmean_sb")
            if PHASE >= 4:
                nc.vector.tensor_scalar(gmean_sb[:], gsum_ps[:GPC, :],
                                        invcntc[:, :], None, AOP.mult)
            else:
                nc.vector.memset(gmean_sb[:], 0.0)
            nc.sync.dma_start(gpart[:, 0:H], gmean_sb[:])
            # max: padded per-graph gather from pool_tab + transpose + reduce
            MAXN = p.MAXN
            CPG = MAXN // 128
            GPCALL = 1024 // MAXN            # graphs per gather call
            if PHASE < 0:
                nc.vector.memset(gmaxT[:], 0.0)
            for j0 in (range(0, GPC, GPCALL) if PHASE >= 0 else []):
                j1 = min(j0 + GPCALL, GPC)
                nidx = (j1 - j0) * MAXN
                pg = msgp.tile([128, MAXG // 128, 128], BF, tag="msg")
                nc.gpsimd.dma_gather(
                    out_ap=pg[:, :nidx // 128, :],
                    in_ap=pool_tab[:],
                    idxs_ap=pmax_sb[:, j0 * MAXN // 16:
                                    j0 * MAXN // 16 + nidx // 16],
                    num_idxs=nidx, num_idxs_reg=nidx,
                    elem_size=H, queue_num=next_q())
                for j in range(j0, j1):
                    tp = psm.tile([128, 128 * CPG], BF, tag="sm")
                    for cc in range(CPG):
                        ch = pg[:, (j - j0) * CPG + cc, :]
                        nc.tensor.transpose(tp[:, cc * 128:(cc + 1) * 128],
                                            ch, ident_bf[:, :])
                    nc.vector.reduce_max(gmaxT[:, j:j + 1], tp[:], axis=AXX)
            tp = psm.tile([128, 128], F32, tag="sm")
            nc.tensor.transpose(tp[:GPC, :], gmaxT[:, :], ident_f[:, :])
            gsb = bp.tile([GPC, 128], F32, tag="gsb")
            nc.vector.tensor_copy(gsb[:], tp[:GPC, :])
            nc.sync.dma_start(gpart[:, H:2 * H], gsb[:])
            nc.gpsimd.collective_compute(
                "AllGather", AOP.bypass,
                replica_groups=[list(range(NCORES))],
                ins=[gpart[:].opt()], outs=[gfull[:].opt()])

            # =============== head ===============
            for t in range(NGRAPH // 128):
                gt = bp.tile([128, 2 * H], F32, tag="gt")
                nc.sync.dma_start(gt[:], gfull[t * 128:(t + 1) * 128, :])
                h1_ps = pmm.tile([128, 512], F32, tag="mm")
                for half in range(2):
                    gT_ps = psm.tile([128, 128], F32, tag="sm")
                    nc.tensor.transpose(gT_ps[:], gt[:, half * H:(half + 1) * H],
                                        ident_f[:, :])
                    gT = bp.tile([128, 128], F32, tag="gT")
                    nc.vector.tensor_copy(gT[:], gT_ps[:])
                    nc.tensor.matmul(h1_ps[:, :H], lhsT=gT[:],
                                     rhs=hW1[:, half * H:(half + 1) * H],
                                     start=(half == 0), stop=(half == 1),
                                     skip_group_check=True)
                h1 = bp.tile([128, H], F32, tag="h1")
                if any_hb1:
                    nc.vector.tensor_tensor(
                        out=h1[:], in0=h1_ps[:, :H],
                        in1=rows[:, 13 * H:14 * H], op=AOP.add)
                    nc.vector.tensor_scalar(h1[:], h1[:], 0.0, None, AOP.max)
                else:
                    nc.vector.tensor_scalar(h1[:], h1_ps[:, :H], 0.0, None,
                                            AOP.max)
                h1T_ps = psm.tile([128, 128], F32, tag="sm")
                nc.tensor.transpose(h1T_ps[:], h1[:], ident_f[:, :])
                h1T = bp.tile([128, 128], F32, tag="h1T")
                nc.vector.tensor_copy(h1T[:], h1T_ps[:])
                o_ps = pmm.tile([128, 512], F32, tag="mm")
                nc.tensor.matmul(o_ps[:, :NCLS], lhsT=h1T[:], rhs=hW2[:, :],
                                 start=True, stop=True)
                o = bp.tile([128, NCLS], F32, tag="o")
                if any_hb2:
                    nc.vector.tensor_tensor(
                        out=o[:], in0=o_ps[:, :NCLS],
                        in1=rows[:, 14 * H:14 * H + NCLS], op=AOP.add)
                else:
                    nc.vector.tensor_copy(o[:], o_ps[:, :NCLS])
                nc.sync.dma_start(t_out[t * 128:(t + 1) * 128, :], o[:])

    nc.compile()
    _fix_act_tables(nc)
    _split_waits(nc)
    return nc


def make_in_maps(p, w):
    iota = np.tile(np.arange(128, dtype=np.float32), (128, 1))
    rows = np.zeros((16, H), np.float32)
    for i in range(4):
        rows[i] = np.asarray(w[f'cb{i}'], np.float32)
        rows[4 + i] = np.asarray(w[f'g{i}'], np.float32)
        rows[8 + i] = np.asarray(w[f'be{i}'], np.float32)
    rows[12] = np.asarray(w['res_b'], np.float32)
    rows[13] = np.asarray(w['hb1'], np.float32)
    rows[14, :NCLS] = np.asarray(w['hb2'], np.float32)
    hW1 = np.asarray(w['hW1'], np.float32)          # [256, 128]
    hW1_pack = np.concatenate([hW1[:H, :], hW1[H:, :]], axis=1)  # [128, 256]
    Wk_pack = np.concatenate(
        [np.asarray(w[f'W{i}'], np.float32) for i in (1, 2, 3)], axis=1)
    shared = {
        "W0": np.asarray(w['W0'], np.float32),
        "resW": np.asarray(w['res_W'], np.float32),
        "Wk": Wk_pack.astype(BF16),
        "rows": np.tile(rows.reshape(1, 16 * H), (128, 1)),
        "eeW1": np.asarray(w['ee_W1'], np.float32).astype(BF16),
        "eeW2": np.asarray(w['ee_W2'], np.float32).astype(BF16),
        "eeb1": np.asarray(w['ee_b1'], np.float32).reshape(H, 1),
        "hW1": hW1_pack,
        "hW2": np.asarray(w['hW2'], np.float32),
        "iota_bf": iota.astype(BF16),
        "ident_bf": np.eye(128, dtype=np.float32).astype(BF16),
        "ident_f": np.eye(128, dtype=np.float32),
    }
    in_maps = []
    for r in range(NCORES):
        m = dict(shared)
        m.update({
            "xT": p.xT[r], "idx": p.idx_all[r], "dstloc": p.dstloc_all[r],
            "eaT": p.eaT_all[r],
            "invcntc": p.invcntc[r], "gmemb": p.gmemb[r],
            "pmax": p.pmax_idx[r],
        })
        in_maps.append(m)
    return in_maps


def kernel(**inputs):
    from concourse.bass_utils import run_bass_kernel_spmd
    p = make_plan(inputs['x'], inputs['edge_index'], inputs['batch'],
                  inputs['edge_attr'])
    nc = build_nc(p, inputs)
    in_maps = make_in_maps(p, inputs)
    res = run_bass_kernel_spmd(nc, in_maps, core_ids=list(range(NCORES)),
                               trace=False)
    return np.asarray(res.results[0]["out"], np.float32).copy()
